# revision 16
# baseline (speedup 1.0000x reference)
"""HGConv fused kernel for one TRN2 chip (8 NeuronCores), SPMD via Bass/Tile.

Hardcoded for M=16384 nodes, E=4096 hyperedges, D=300, N_CAT=3, 8 cores.

Strategy (edge-sharded, zero collectives):
  - Core c owns hyperedges [512c, 512(c+1)).  It loads the FULL node
    matrix X (16384, 300) plus its own 512-column slice of inc, both in
    fp16 (halves HBM traffic; final rel err ~1e-3), and computes
    IX = inc_c.T @ X with the full m=16384 contraction locally — no
    ReduceScatter, no AllGather, no cross-core barrier at all.
  - Host pre-tiles both operands to [p, ...] layout so every DMA is a
    contiguous multi-KB line per partition.  inc streams et-major in
    32-m-tile chunks round-robined over three DMA queues (sync joins
    once X is down); small leading chunks start the PE by ~12 us.
  - Per-e-tile tail is software-pipelined two e-tiles deep behind the
    accumulation matmuls: early (transposes, edge_att = IX @ W_att,
    softmax over d, ef = IX*attn), late1 (ef @ W_proj + residual),
    late2 (edge scores via fused multiply-reduce against a broadcast
    ec_W_att, unstabilized exp — |score| < 5 — and the pooling matmul
    p2z = expw.T @ [ef2 | 1], which carries z in its last column).
  - ec_W_proj @ fc_W is folded on the host into one (300, 3) weight.
    Each core outputs 8 floats: [r(3), z, pad]; the host sums the 8
    partials (global softmax over edges = sum of unnormalized local
    partials) and adds ec_b @ fc_W + fc_b.
"""

import sys

for _p in ("/opt/trn_rl_repo", "/opt/pypackages"):
    if _p not in sys.path:
        sys.path.append(_p)

import numpy as np

import concourse.bacc as bacc
import concourse.tile as tile
from concourse import masks, mybir
from concourse.bass_utils import run_bass_kernel_spmd

F32 = mybir.dt.float32
F16 = mybir.dt.float16
BF16 = mybir.dt.bfloat16
AX = mybir.AxisListType
OP = mybir.AluOpType
AF = mybir.ActivationFunctionType

NCORES = 8
M, E, D, NCAT = 16384, 4096, 300, 3
E_SH = E // NCORES          # 512 edges per core
ET = E_SH // 128            # 4 e-tiles per core
T = M // 128                # 128 m-tiles (full contraction on every core)
DCH = (128, 128, 44)        # d split into partition chunks
DOF = (0, 128, 256)
SLABS = [(0, 16), (16, 32), (32, 64), (64, 96)]     # phase A t-slabs
B_LO, B_HI = 96, 128                                # phase B t-range


def _build(alpha: float, mode: str):
    nc = bacc.Bacc("TRN2", target_bir_lowering=False, debug=False,
                   num_devices=NCORES)
    in_dt = F16 if mode == "f16" else BF16
    x_d = nc.dram_tensor("x", [128, T, D], in_dt, kind="ExternalInput")
    inc_d = nc.dram_tensor("inc", [128, ET, T, 128], in_dt,
                           kind="ExternalInput")
    ef_d = nc.dram_tensor("efeat", [E_SH, D], F32, kind="ExternalInput")
    watt_d = nc.dram_tensor("watt", [D, D], in_dt, kind="ExternalInput")
    wproj_d = nc.dram_tensor("wproj", [D, D], in_dt, kind="ExternalInput")
    ecwb_d = nc.dram_tensor("ecwb", [128, D], F32, kind="ExternalInput")
    out_d = nc.dram_tensor("out", [1, 304], F32, kind="ExternalOutput")

    def mm(out, lhsT, rhs, start, stop):
        nc.tensor.matmul(out, lhsT, rhs, start=start, stop=stop)

    with tile.TileContext(nc) as tc, \
         tc.tile_pool(name="sb", bufs=1) as sb, \
         tc.tile_pool(name="xp", bufs=1) as xp, \
         tc.tile_pool(name="incp", bufs=8) as incp, \
         tc.tile_pool(name="pacc", bufs=1, space="PSUM") as pacc, \
         tc.tile_pool(name="pp", bufs=3, space="PSUM") as pp, \
         tc.tile_pool(name="pfix", bufs=1, space="PSUM") as pfix:

        # ---------- deadline-ordered DMA schedule over 3 queues ----------
        # Pieces are emitted in the order the PE will need them (t-slab
        # major in phase A, then phase-B inc + tail weights), round-robin
        # across the three DMA-capable engines so no single queue's
        # ~130 GB/s serializes the critical path.
        qs = (nc.sync, nc.scalar, nc.gpsimd)
        qn = [0]

        def issue(fn):
            fn(qs[qn[0] % 3])
            qn[0] += 1

        x_tiles = {}
        inc_tiles = {}

        def x_piece(lo, hi):
            def go(eng):
                xt = xp.tile([128, hi - lo, D], in_dt, tag=f"x{lo}",
                             name=f"x{lo}")
                eng.dma_start(xt[:], x_d[:, lo:hi, :])
                x_tiles[lo] = xt
            return go

        def inc_piece(et, lo, hi):
            def go(eng):
                it = incp.tile([128, 32, 128], in_dt, tag="inc",
                               name=f"inc{et}_{lo}")
                eng.dma_start(it[:, 0:hi - lo, :], inc_d[:, et, lo:hi, :])
                inc_tiles[(et, lo)] = it
            return go

        def w_piece(dst, src):
            def go(eng):
                eng.dma_start(dst, src)
            return go

        watt_sb = sb.tile([128, 3, D], in_dt)
        wproj_sb = sb.tile([128, 3, D], in_dt)
        efeat_sb = sb.tile([128, ET, D], F32)   # pre-scaled by alpha on host
        ecwb_sb = sb.tile([128, D], F32)        # ec_W_att broadcast to 128 p

        for lo, hi in SLABS:                    # phase A slabs
            mid = (lo + hi) // 2
            issue(x_piece(lo, mid))
            issue(x_piece(mid, hi))
            for et in range(ET):
                issue(inc_piece(et, lo, hi))
        for i, (c, o) in enumerate(zip(DCH, DOF)):
            issue(w_piece(watt_sb[:c, i, :], watt_d[o:o + c, :]))
        issue(x_piece(B_LO, B_LO + 16))         # phase B
        issue(x_piece(B_LO + 16, B_HI))
        for et in range(ET):
            issue(inc_piece(et, B_LO, B_HI))
        for i, (c, o) in enumerate(zip(DCH, DOF)):
            issue(w_piece(wproj_sb[:c, i, :], wproj_d[o:o + c, :]))
        issue(w_piece(efeat_sb[:],
                      ef_d.ap().rearrange("(t p) d -> p t d", p=128)))
        issue(w_piece(ecwb_sb[:], ecwb_d[:]))
        ident = sb.tile([128, 128], F32)
        masks.make_identity(nc, ident[:])

        x_offs = sorted(x_tiles)

        def x_at(t):
            lo = max(o for o in x_offs if o <= t)
            return x_tiles[lo][:, t - lo, :]

        # ---------- working tiles for the tail ----------
        ix_sb = sb.tile([128, ET, D], F32)
        ef_sb = sb.tile([128, ET, D], F32)
        ef2z_sb = sb.tile([128, ET, 304], F32)  # [:, :, 300] = 1.0 (z col)
        nc.vector.memset(ef2z_sb[:, :, 300:301], 1.0)
        ixT_sb = sb.tile([128, 3, E_SH], in_dt)
        efT_sb = sb.tile([128, 3, E_SH], in_dt)
        stat_sb = sb.tile([128, ET, 4], F32)
        tmp_sb = sb.tile([128, D], F32)
        expcol_sb = sb.tile([128, ET], F32)
        p2z = pfix.tile([1, 304], F32, tag="p2", name="p2z")

        def tr_chunks(src_sb, dstT_sb, et, use_vec):
            # (128e, 300d) -> 3 d-part chunks of (c, 128e), cast to in_dt
            for i, (c, o) in enumerate(zip(DCH, DOF)):
                tp = pp.tile([128, 128], F32, tag="ps", name=f"tp{et}_{i}")
                nc.tensor.transpose(tp[:c, :128], src_sb[:, et, o:o + c],
                                    ident[:])
                dst = dstT_sb[:c, i, et * 128:(et + 1) * 128]
                if use_vec:
                    nc.vector.tensor_copy(dst, tp[:c, :128])
                else:
                    nc.scalar.copy(dst, tp[:c, :128])

        def early_tail(et):
            # IX psum -> sbuf; edge_att = IX @ W_att; softmax over d;
            # ef = IX * attn
            nc.vector.tensor_copy(ix_sb[:, et, :], accs[et][:])
            tr_chunks(ix_sb, ixT_sb, et, False)
            att = pp.tile([128, D], F32, tag="ps", name=f"att{et}")
            for i, c in enumerate(DCH):
                mm(att[:], ixT_sb[:c, i, et * 128:(et + 1) * 128],
                   watt_sb[:c, i, :], start=(i == 0), stop=(i == 2))
            nmax = stat_sb[:, et, 0:1]
            nc.vector.tensor_reduce(nmax, att[:], axis=AX.X, op=OP.max,
                                    negate=True)
            ex = pp.tile([128, D], F32, tag="ps", name=f"ex{et}")
            rsum = stat_sb[:, et, 1:2]
            nc.scalar.activation(ex[:], att[:], AF.Exp, bias=nmax,
                                 scale=1.0, accum_out=rsum)
            rcp = stat_sb[:, et, 2:3]
            nc.vector.reciprocal(rcp, rsum)
            nc.vector.scalar_tensor_tensor(
                ef_sb[:, et, :], ex[:], rcp, ix_sb[:, et, :],
                op0=OP.mult, op1=OP.mult)

        def late1(et):
            # ef2 = alpha*edge_feats + (1-alpha)*(ef @ W_proj)
            tr_chunks(ef_sb, efT_sb, et, True)
            prj = pp.tile([128, D], F32, tag="ps", name=f"prj{et}")
            for i, c in enumerate(DCH):
                mm(prj[:], efT_sb[:c, i, et * 128:(et + 1) * 128],
                   wproj_sb[:c, i, :], start=(i == 0), stop=(i == 2))
            nc.vector.scalar_tensor_tensor(
                ef2z_sb[:, et, 0:D], prj[:], float(1.0 - alpha),
                efeat_sb[:, et, :], op0=OP.mult, op1=OP.add)

        def late2(et):
            # edge scores via fused row-dot; unstabilized exp (|sc| < 5);
            # p2z += expw_et^T @ [ef2_et | 1]
            sccol = stat_sb[:, et, 3:4]
            nc.vector.tensor_tensor(tmp_sb[:], ef2z_sb[:, et, 0:D],
                                    ecwb_sb[:], op=OP.mult)
            nc.vector.tensor_reduce(sccol, tmp_sb[:], axis=AX.X, op=OP.add)
            nc.scalar.activation(expcol_sb[:, et:et + 1], sccol, AF.Exp)
            mm(p2z[:, 0:301], expcol_sb[:, et:et + 1], ef2z_sb[:, et, 0:301],
               start=(et == 0), stop=(et == ET - 1))

        accs = [pacc.tile([128, D], F32, tag=f"a{et}", name=f"acc{et}")
                for et in range(ET)]

        # ---------- phase A: t-slab major, all 4 e-tiles per slab ----------
        # PE consumption rate matches the combined x+inc arrival rate, so
        # the accumulation streams without starvation gaps.
        for lo, hi in SLABS:
            for tt in range(lo, hi):
                for et in range(ET):
                    mm(accs[et][:], inc_tiles[(et, lo)][:, tt - lo, :],
                       x_at(tt), start=(tt == 0), stop=False)

        # ---------- phase B: finish each e-tile, 2-deep pipelined tail ----
        for et in range(ET):
            it = inc_tiles[(et, B_LO)]
            for tt in range(B_LO, B_HI):
                mm(accs[et][:], it[:, tt - B_LO, :], x_at(tt),
                   start=False, stop=(tt == B_HI - 1))
            if et >= 1:
                late1(et - 1)
            if et >= 2:
                late2(et - 2)
            early_tail(et)
        late1(ET - 1)
        late2(ET - 2)
        late2(ET - 1)

        # ---------- ship the pooled partial [p2(300) | z | pad] ----------
        out_sb = sb.tile([1, 304], F32)
        nc.vector.memset(out_sb[:, 301:304], 0.0)
        nc.scalar.copy(out_sb[:, 0:301], p2z[:, 0:301])
        nc.sync.dma_start(out_d[:], out_sb[:])

    nc.compile()
    return nc


_CACHE = {}


def get_nc(alpha: float, mode: str = "f16"):
    key = (alpha, mode)
    if key not in _CACHE:
        _CACHE[key] = _build(alpha, mode)
    return _CACHE[key]


def make_in_maps(node_feats, edge_feats, inc_mat, W_att, W_proj, alpha,
                 ec_W_att, ec_W_proj, fc_W, mode="f16"):
    if mode == "f16":
        ndt = np.float16
    else:
        import ml_dtypes
        ndt = ml_dtypes.bfloat16
    wdt = lambda a: np.ascontiguousarray(np.asarray(a, np.float32).astype(ndt))
    x = np.asarray(node_feats, np.float32).astype(ndt)
    xt = np.ascontiguousarray(x.reshape(T, 128, D).transpose(1, 0, 2))
    inc = np.asarray(inc_mat, np.float32).astype(ndt)
    ef_scaled = np.asarray(edge_feats, np.float32) * np.float32(alpha)
    ecwb = np.ascontiguousarray(np.broadcast_to(
        np.asarray(ec_W_att, np.float32).reshape(1, D), (128, D)))
    common = dict(x=xt, watt=wdt(W_att), wproj=wdt(W_proj), ecwb=ecwb)
    in_maps = []
    for c in range(NCORES):
        inc_c = inc[:, c * E_SH:(c + 1) * E_SH]
        # [p, et, t, e8]: m = t*128 + p, e_local = et*128 + e8
        inc_t = np.ascontiguousarray(
            inc_c.reshape(T, 128, ET, 128).transpose(1, 2, 0, 3))
        in_maps.append(dict(
            inc=inc_t,
            efeat=np.ascontiguousarray(ef_scaled[c * E_SH:(c + 1) * E_SH]),
            **common))
    return in_maps


def kernel(node_feats, edge_feats, inc_mat, W_att, W_proj, alpha,
           ec_W_att, ec_W_proj, ec_b_proj, fc_W, fc_b,
           mode="f16", trace=False):
    alpha_f = float(np.asarray(alpha))
    nc = get_nc(alpha_f, mode)
    in_maps = make_in_maps(node_feats, edge_feats, inc_mat, W_att, W_proj,
                           alpha_f, ec_W_att, ec_W_proj, fc_W, mode=mode)
    res = run_bass_kernel_spmd(nc, in_maps, list(range(NCORES)), trace=trace)
    kernel.last_results = res
    outs = np.stack([np.asarray(res.results[c]["out"]).reshape(304)
                     for c in range(NCORES)]).astype(np.float64)
    pooled = outs[:, 0:D].sum(axis=0) / outs[:, D].sum()
    out = pooled @ np.asarray(ec_W_proj, np.float64) + np.asarray(
        ec_b_proj, np.float64)
    logits = out @ np.asarray(fc_W, np.float64) + np.asarray(fc_b, np.float64)
    return logits.astype(np.float32)
